# revision 21
# baseline (speedup 1.0000x reference)
"""Causal multi-head attention on 8 Trainium2 NeuronCores.

Sharding: core c handles batch b = c//2 and head-half hg = c%2 (8 of 16
heads, as 4 pairs). Per core: QKV projection (bf16 matmuls, f32 PSUM),
flash-style causal attention in transposed layout (scores_T[t, s], softmax
denominator via a ones-column appended to V), pairwise AllGather of the
normalized attention outputs, and a column-parallel output projection
(w_o columns sharded host-side per core parity). Host reassembles y from
the per-core [m_half, s] transposed outputs.

Loop structure: s-tiles outer; each s-tile's attention is interleaved with
the next s-tile's QKV-projection matmuls and the previous s-tile's output
projection so TensorE stays dense while ScalarE runs the exps, and the
pairwise AllGather per s-tile hides under the next s-tile's compute.
"""
import sys

sys.path.insert(0, "/opt/trn_rl_repo")

import numpy as np
import ml_dtypes

import concourse.bass as bass
import concourse.mybir as mybir
import concourse.tile as tile
from concourse import bacc
from concourse.bass_utils import run_bass_kernel_spmd

BF16 = ml_dtypes.bfloat16
DT = mybir.dt.bfloat16
F32 = mybir.dt.float32
EXP = mybir.ActivationFunctionType.Exp

B, S, DM, H, DK = 4, 2048, 1024, 16, 64
N_CORES = 8
N_PAIRS = 4          # head pairs per core (8 heads)
N_MCH = DM // 128    # m-chunks of the model dim (contraction for QKV proj)
REPLICA_GROUPS = [[0, 1], [2, 3], [4, 5], [6, 7]]


def build_nc(seq=S, n_pairs=N_PAIRS, debug_taps=False):
    """Build the SPMD kernel graph. seq must be a multiple of 512."""
    nst = seq // 512          # 512-wide s-tiles
    ntt_all = seq // 128      # 128-wide t-tiles
    nc = bacc.Bacc("TRN2", target_bir_lowering=False, debug=False,
                   num_devices=N_CORES)

    xT = nc.dram_tensor("xT", [DM, seq], DT, kind="ExternalInput")
    wq = nc.dram_tensor("wq", [DM, 128 * n_pairs], DT, kind="ExternalInput")
    wk = nc.dram_tensor("wk", [DM, 128 * n_pairs], DT, kind="ExternalInput")
    wv = nc.dram_tensor("wv", [DM, 128 * n_pairs], DT, kind="ExternalInput")
    wo = nc.dram_tensor("wo", [2 * 128 * n_pairs, 512], DT, kind="ExternalInput")
    mask128 = nc.dram_tensor("mask128", [128, 128], DT, kind="ExternalInput")
    yT = nc.dram_tensor("yT", [512, seq], F32, kind="ExternalOutput")

    n_dch = 2 * n_pairs   # d-chunks of 128 in the gathered attention
    hw = 128 * n_pairs    # head-dim columns per core (2*n_pairs heads x 64)

    with tile.TileContext(nc) as tc:
        with (
            tc.tile_pool(name="dram", bufs=1, space="DRAM") as dram,
            tc.tile_pool(name="persist", bufs=1) as persist,
            tc.tile_pool(name="psum_p", bufs=1, space="PSUM") as pp,
            tc.tile_pool(name="psum_s", bufs=2, space="PSUM") as ps_s,
            tc.tile_pool(name="psum_av", bufs=3, space="PSUM") as ps_av,
            tc.tile_pool(name="pt", bufs=6) as p_pool,
            tc.tile_pool(name="nrm", bufs=2) as nrm,
            tc.tile_pool(name="yc", bufs=2) as ycp,
            tc.tile_pool(name="stg", bufs=2) as stg,
        ):
            ag_in = dram.tile([nst, 2, 64, n_pairs, 512], DT)
            ag_out = dram.tile([nst, 2, 2, 64, n_pairs, 512], DT)
            ag_in_l = dram.tile([n_pairs, 2, 64, 512], DT)
            ag_out_l = dram.tile([n_pairs, 2, 2, 64, 512], DT)

            q_sb = persist.tile([128, n_pairs, seq], DT, tag="q")
            k_sb = persist.tile([128, n_pairs, seq], DT, tag="k")
            v_sb = persist.tile([128, ntt_all, 2 * n_pairs, 65], DT, tag="v")
            af_sb = persist.tile([128, n_dch, max(seq - 512, 512)], DT, tag="af")
            af3 = []
            for p in range(n_pairs):
                t3 = persist.tile([128, 2, 512], DT, tag=f"af3_{p}")
                af3.append(t3)
            m_sb = persist.tile([128, 128], DT, tag="m")
            wo_sb = persist.tile([128, n_dch, 512], DT, tag="wo")
            wq_sb = persist.tile([128, N_MCH, hw], DT, tag="wq")
            wk_sb = persist.tile([128, N_MCH, hw], DT, tag="wk")
            wv_sb = persist.tile([128, N_MCH, hw], DT, tag="wv")
            xt = []
            for st in range(nst):
                t = persist.tile([128, N_MCH, 512], DT, tag=f"xt{st}")
                xt.append(t)

            xT_v = xT[:].rearrange("(c p) s -> p c s", p=128)
            # spread input loads across per-engine DMA queues so the
            # critical tiles (xt0, wv, wq) land concurrently
            nc.sync.dma_start(out=xt[0][:], in_=xT_v[:, :, 0:512])
            nc.scalar.dma_start(
                out=wv_sb[:], in_=wv[:].rearrange("(c p) n -> p c n", p=128))
            nc.gpsimd.dma_start(
                out=wq_sb[:], in_=wq[:].rearrange("(c p) n -> p c n", p=128))
            nc.scalar.dma_start(
                out=wk_sb[:], in_=wk[:].rearrange("(c p) n -> p c n", p=128))
            if nst > 1:
                nc.gpsimd.dma_start(
                    out=xt[1][:], in_=xT_v[:, :, 512:1024])
            for st in range(2, nst):
                nc.sync.dma_start(
                    out=xt[st][:], in_=xT_v[:, :, st * 512:(st + 1) * 512])
            nc.gpsimd.dma_start(out=m_sb[:], in_=mask128[:])
            nc.sync.dma_start(
                out=wo_sb[:], in_=wo[:].rearrange("(c p) n -> p c n", p=128))
            nc.vector.memset(v_sb[:, :, :, 64], 1.0)
            # PE warm-up during the input-load window: dummy matmuls on a
            # memset tile keep the HAM busy-window open (no DMA dependency)
            warm = persist.tile([128, 512], DT, tag="warm")
            nc.vector.memset(warm[:], 0.0)
            for wi in range(3):
                wps = ps_s.tile([128, 2, 512], F32, tag="sc",
                                name=f"warm{wi}")
                for wj in range(12):
                    nc.tensor.matmul(
                        wps[:, wj % 2, :],
                        lhsT=warm[:, 0:128], rhs=warm[:],
                        start=True, stop=True)

            yT_v = yT[:].rearrange("(t p) s -> p t s", p=128)

            # ---- emission helpers (each returns a closure doing one
            # PE-dense psum-group; used to fill PE during attention) ----
            def vproj_group(tt):
                def go():
                    st, r = tt // 4, tt % 4
                    ps = pp.tile([128, hw], F32, tag="proj", name=f"psv{tt}")
                    for c in range(N_MCH):
                        nc.tensor.matmul(
                            ps[:],
                            lhsT=xt[st][:, c, r * 128:(r + 1) * 128],
                            rhs=wv_sb[:, c, 0:hw],
                            start=(c == 0), stop=(c == N_MCH - 1))
                    nc.any.tensor_copy(
                        v_sb[:, tt, :, 0:64],
                        ps[:].rearrange("p (h k) -> p h k", k=64))
                return go

            def qkproj_group(pair, st, which):
                def go():
                    w_sb, dst = ((wq_sb, q_sb), (wk_sb, k_sb))[which]
                    ps = pp.tile([128, 512], F32, tag="proj",
                                 name=f"psqk{pair}_{st}_{which}")
                    for c in range(N_MCH):
                        nc.tensor.matmul(
                            ps[:],
                            lhsT=w_sb[:, c, pair * 128:(pair + 1) * 128],
                            rhs=xt[st][:, c, :],
                            start=(c == 0), stop=(c == N_MCH - 1))
                    nc.any.tensor_copy(
                        dst[:, pair, st * 512:(st + 1) * 512], ps[:])
                return go

            def outproj_group(mt, st, chunk_order=None):
                def go():
                    ps = pp.tile([128, 512], F32, tag="proj",
                                 name=f"pso{mt}_{st}")
                    order = chunk_order or list(range(n_dch))
                    last = st == nst - 1
                    for i, c in enumerate(order):
                        g, p = c // n_pairs, c % n_pairs
                        rhs = (af3[p][:, g, :] if last
                               else af_sb[:, c, st * 512:(st + 1) * 512])
                        nc.tensor.matmul(
                            ps[:],
                            lhsT=wo_sb[:, c, mt * 128:(mt + 1) * 128],
                            rhs=rhs,
                            start=(i == 0), stop=(i == n_dch - 1))
                    yc = ycp.tile([128, 512], F32, tag="yc", name=f"yc{mt}_{st}")
                    nc.any.tensor_copy(yc[:], ps[:])
                    nc.sync.dma_start(
                        out=yT_v[:, mt, st * 512:(st + 1) * 512], in_=yc[:])
                return go

            def proj_groups_for_st(st):
                gs = []
                for tt in range(4 * st, 4 * st + 4):
                    gs.append(vproj_group(tt))
                for pair in range(n_pairs):
                    for which in range(2):
                        gs.append(qkproj_group(pair, st, which))
                return gs

            if debug_taps:
                dpt = nc.dram_tensor("dpt", [4, 128, 2, 512], DT,
                                     kind="ExternalOutput")
                dav = nc.dram_tensor("dav", [2, 65, 512], F32,
                                     kind="ExternalOutput")
                dr = nc.dram_tensor("dr", [1, 2, 512], F32,
                                    kind="ExternalOutput")
                dbb = nc.dram_tensor("dbb", [64, 2, 512], F32,
                                     kind="ExternalOutput")

            # ---- attention for one (pair, st), software-pipelined ----
            def attention(pair, st, filler, stage, pace):
                ntt = 4 * st + 4
                av0 = ps_av.tile([65, 512], F32, tag="av",
                                 name=f"av0_{pair}_{st}")
                av1 = ps_av.tile([65, 512], F32, tag="av",
                                 name=f"av1_{pair}_{st}")
                av = [av0, av1]
                pts = {}

                def scores_and_exp(tt):
                    ps = ps_s.tile([128, 2, 512], F32, tag="sc",
                                   name=f"sc{pair}_{st}_{tt}")
                    for h in range(2):
                        lo = h * 64
                        nc.tensor.matmul(
                            ps[:, h, :],
                            lhsT=k_sb[lo:lo + 64, pair,
                                      tt * 128:(tt + 1) * 128],
                            rhs=q_sb[lo:lo + 64, pair,
                                     st * 512:(st + 1) * 512],
                            start=True, stop=True)
                    pt = p_pool.tile([128, 2, 512], DT, tag="pt",
                                     name=f"pt{pair}_{st}_{tt}")
                    kk = tt - 4 * st
                    if kk < 0:
                        nc.scalar.activation(pt[:], ps[:], EXP, scale=0.125)
                    else:
                        # diagonal: zero the fully-masked cols, exp the rest,
                        # triangular mask on the boundary 128-col block
                        nc.scalar.activation(
                            pt[:, :, kk * 128:512],
                            ps[:, :, kk * 128:512], EXP, scale=0.125)
                        for h in range(2):
                            nc.vector.tensor_mul(
                                pt[:, h, kk * 128:(kk + 1) * 128],
                                pt[:, h, kk * 128:(kk + 1) * 128],
                                m_sb[:])
                    if debug_taps and pair == 0 and st == 0:
                        nc.sync.dma_start(out=dpt[tt], in_=pt[:])
                    pts[tt] = pt

                def pv(tt):
                    pt = pts.pop(tt)
                    kk = tt - 4 * st
                    f0 = kk * 128 if kk > 0 else 0
                    for h in range(2):
                        nc.tensor.matmul(
                            av[h][:, f0:512],
                            lhsT=v_sb[:, tt, 2 * pair + h, :],
                            rhs=pt[:, h, f0:512],
                            start=(tt == 0), stop=(tt == ntt - 1))

                for tt in range(ntt + 1):
                    if tt < ntt:
                        scores_and_exp(tt)
                    if tt > 0:
                        pv(tt - 1)
                    pace["done"] += 1
                    owed = (pace["pops"] * pace["done"]) // pace["total"] \
                        - pace["popped"]
                    while filler and owed > 0:
                        filler.pop(0)()
                        pace["popped"] += 1
                        owed -= 1

                if debug_taps and pair == 0 and st == 0:
                    for h in range(2):
                        avc = nrm.tile([65, 512], F32, tag="avc",
                                       name=f"avc{h}")
                        nc.vector.tensor_copy(avc[:], av[h][:])
                        nc.sync.dma_start(out=dav[h], in_=avc[:])
                # normalize: a = av[0:64] * (1/denom); denom row (psum
                # partition 64) -> sbuf -> DMA to partition 0 (the custom-DVE
                # recip and gpsimd broadcast only read partition 0 correctly)
                den = nrm.tile([65, 2, 512], DT, tag="den",
                               name=f"den{pair}_{st}")
                for h in range(2):
                    nc.vector.tensor_copy(den[64:65, h, :], av[h][64:65, :])
                den0 = nrm.tile([1, 2, 512], F32, tag="den0",
                                name=f"den0_{pair}_{st}")
                nc.gpsimd.dma_start(out=den0[:], in_=den[64:65, :, :])
                r = nrm.tile([1, 2, 512], F32, tag="r", name=f"r{pair}_{st}")
                nc.vector.reciprocal_approx_fast(r[:], den0[:])
                bb = nrm.tile([64, 2, 512], F32, tag="b", name=f"bb{pair}_{st}")
                nc.gpsimd.partition_broadcast(bb[:], r[:])
                if debug_taps and pair == 0 and st == 0:
                    nc.sync.dma_start(out=dr[:], in_=r[:])
                    nc.sync.dma_start(out=dbb[:], in_=bb[:])
                for h in range(2):
                    nc.vector.tensor_mul(
                        stage[:, h, pair, :],
                        av[h][0:64, :], bb[:, h, :])

            # ---------------- main s-tile-outer schedule ----------------
            pending = proj_groups_for_st(0)
            while pending:
                pending.pop(0)()
            for st in range(nst):
                filler = []
                if st + 1 < nst:
                    filler += proj_groups_for_st(st + 1)
                if st >= 1:
                    for mt in range(4):
                        filler.append(outproj_group(mt, st - 1))
                stage = stg.tile([64, 2, n_pairs, 512], DT, tag="stage",
                                 name=f"stage{st}")
                total_iters = n_pairs * (4 * st + 5)
                pace = {"total": total_iters, "done": 0,
                        "pops": len(filler), "popped": 0}
                for pair in range(n_pairs):
                    attention(pair, st, filler, stage, pace)
                    if st == nst - 1:
                        for h in range(2):
                            nc.sync.dma_start(
                                out=ag_in_l[pair, h],
                                in_=stage[:, h, pair, :])
                        nc.gpsimd.collective_compute(
                            "AllGather",
                            mybir.AluOpType.bypass,
                            replica_groups=REPLICA_GROUPS,
                            ins=[ag_in_l[pair].opt()],
                            outs=[ag_out_l[pair].opt()],
                        )
                        for g in range(2):
                            for h in range(2):
                                nc.sync.dma_start(
                                    out=af3[pair][h * 64:(h + 1) * 64, g, :],
                                    in_=ag_out_l[pair, g, h])
                while filler:
                    filler.pop(0)()
                # exchange this s-tile's attention columns
                if st < nst - 1:
                    for h in range(2):
                        nc.sync.dma_start(
                            out=ag_in[st, h], in_=stage[:, h, :, :])
                    nc.gpsimd.collective_compute(
                        "AllGather",
                        mybir.AluOpType.bypass,
                        replica_groups=REPLICA_GROUPS,
                        ins=[ag_in[st].opt()],
                        outs=[ag_out[st].opt()],
                    )
                    for g in range(2):
                        for h in range(2):
                            nc.sync.dma_start(
                                out=af_sb[h * 64:(h + 1) * 64,
                                          g * n_pairs:(g + 1) * n_pairs,
                                          st * 512:(st + 1) * 512],
                                in_=ag_out[st, g, h])
            pair_major = [g * n_pairs + p for p in range(n_pairs)
                          for g in range(2)]
            for mt in range(4):
                outproj_group(mt, nst - 1, chunk_order=pair_major)()

            if debug_taps:
                dq = nc.dram_tensor("dq", [128, n_pairs, seq], DT,
                                    kind="ExternalOutput")
                dk = nc.dram_tensor("dk", [128, n_pairs, seq], DT,
                                    kind="ExternalOutput")
                dv = nc.dram_tensor("dv", [128, ntt_all, 2 * n_pairs, 65], DT,
                                    kind="ExternalOutput")
                daf = nc.dram_tensor("daf", [128, n_dch, seq], DT,
                                     kind="ExternalOutput")
                for dst, src in ((dq, q_sb), (dk, k_sb), (dv, v_sb),
                                 (daf, af_sb)):
                    nc.sync.dma_start(out=dst[:], in_=src[:])
    nc.compile()
    return nc


def _make_mask128():
    p = np.arange(128)[:, None]
    f = np.arange(128)[None, :]
    return (p <= f).astype(BF16)


_NC_CACHE = {}


def _get_nc(seq=S, n_pairs=N_PAIRS):
    key = (seq, n_pairs)
    if key not in _NC_CACHE:
        _NC_CACHE[key] = build_nc(seq, n_pairs)
    return _NC_CACHE[key]


def make_in_maps(x, w_qkv, w_o):
    masks = _make_mask128()
    in_maps = []
    for c in range(N_CORES):
        b, hg = c // 2, c % 2
        heads = slice(hg * 8, hg * 8 + 8)
        in_maps.append({
            "xT": np.ascontiguousarray(x[b].T).astype(BF16),
            "wq": np.ascontiguousarray(
                w_qkv[0, heads].reshape(512, DM).T).astype(BF16),
            "wk": np.ascontiguousarray(
                w_qkv[1, heads].reshape(512, DM).T).astype(BF16),
            "wv": np.ascontiguousarray(
                w_qkv[2, heads].reshape(512, DM).T).astype(BF16),
            "wo": np.ascontiguousarray(
                w_o[hg * 512:(hg + 1) * 512, :].T).astype(BF16),
            "mask128": masks,
        })
    return in_maps


def kernel(x, w_qkv, w_o):
    x = np.asarray(x, dtype=np.float32)
    w_qkv = np.asarray(w_qkv, dtype=np.float32)
    w_o = np.asarray(w_o, dtype=np.float32)

    nc = _get_nc()
    in_maps = make_in_maps(x, w_qkv, w_o)
    res = run_bass_kernel_spmd(nc, in_maps, list(range(N_CORES)), trace=False)

    y = np.empty((B, S, DM), dtype=np.float32)
    for c in range(N_CORES):
        b, hg = c // 2, c % 2
        y[b, :, hg * 512:(hg + 1) * 512] = res.results[c]["yT"].T
    return y


# revision 22
# speedup vs baseline: 1.0728x; 1.0728x over previous
"""Causal multi-head attention on 8 Trainium2 NeuronCores.

Sharding: core c handles batch b = c//2 and head-half hg = c%2 (8 of 16
heads, as 4 pairs). Per core: QKV projection (bf16 matmuls, f32 PSUM),
flash-style causal attention in transposed layout (scores_T[t, s], softmax
denominator via a ones-column appended to V), pairwise AllGather of the
normalized attention outputs, and a column-parallel output projection
(w_o columns sharded host-side per core parity). Host reassembles y from
the per-core [m_half, s] transposed outputs.

Loop structure: s-tiles outer; each s-tile's attention is interleaved with
the next s-tile's QKV-projection matmuls and the previous s-tile's output
projection so TensorE stays dense while ScalarE runs the exps, and the
pairwise AllGather per s-tile hides under the next s-tile's compute.
"""
import sys

sys.path.insert(0, "/opt/trn_rl_repo")

import numpy as np
import ml_dtypes

import concourse.bass as bass
import concourse.mybir as mybir
import concourse.tile as tile
from concourse import bacc
from concourse.bass_utils import run_bass_kernel_spmd

BF16 = ml_dtypes.bfloat16
DT = mybir.dt.bfloat16
F32 = mybir.dt.float32
EXP = mybir.ActivationFunctionType.Exp

B, S, DM, H, DK = 4, 2048, 1024, 16, 64
N_CORES = 8
N_PAIRS = 4          # head pairs per core (8 heads)
N_MCH = DM // 128    # m-chunks of the model dim (contraction for QKV proj)
REPLICA_GROUPS = [[0, 1], [2, 3], [4, 5], [6, 7]]


def build_nc(seq=S, n_pairs=N_PAIRS, debug_taps=False):
    """Build the SPMD kernel graph. seq must be a multiple of 512."""
    nst = seq // 512          # 512-wide s-tiles
    ntt_all = seq // 128      # 128-wide t-tiles
    nc = bacc.Bacc("TRN2", target_bir_lowering=False, debug=False,
                   num_devices=N_CORES)

    xT = nc.dram_tensor("xT", [DM, seq], DT, kind="ExternalInput")
    wq = nc.dram_tensor("wq", [DM, 128 * n_pairs], DT, kind="ExternalInput")
    wk = nc.dram_tensor("wk", [DM, 128 * n_pairs], DT, kind="ExternalInput")
    wv = nc.dram_tensor("wv", [DM, 128 * n_pairs], DT, kind="ExternalInput")
    wo = nc.dram_tensor("wo", [2 * 128 * n_pairs, 512], DT, kind="ExternalInput")
    mask128 = nc.dram_tensor("mask128", [128, 128], DT, kind="ExternalInput")
    yT = nc.dram_tensor("yT", [512, seq], F32, kind="ExternalOutput")

    n_dch = 2 * n_pairs   # d-chunks of 128 in the gathered attention
    hw = 128 * n_pairs    # head-dim columns per core (2*n_pairs heads x 64)

    with tile.TileContext(nc) as tc:
        with (
            tc.tile_pool(name="dram", bufs=1, space="DRAM") as dram,
            tc.tile_pool(name="persist", bufs=1) as persist,
            tc.tile_pool(name="psum_p", bufs=1, space="PSUM") as pp,
            tc.tile_pool(name="psum_s", bufs=2, space="PSUM") as ps_s,
            tc.tile_pool(name="psum_av", bufs=3, space="PSUM") as ps_av,
            tc.tile_pool(name="pt", bufs=6) as p_pool,
            tc.tile_pool(name="nrm", bufs=2) as nrm,
            tc.tile_pool(name="yc", bufs=2) as ycp,
            tc.tile_pool(name="stg", bufs=2) as stg,
        ):
            ag_in = dram.tile([nst, 2, 64, n_pairs, 512], DT)
            ag_out = dram.tile([nst, 2, 2, 64, n_pairs, 512], DT)
            ag_in_l = dram.tile([n_pairs, 2, 64, 512], DT)
            ag_out_l = dram.tile([n_pairs, 2, 2, 64, 512], DT)

            q_sb = persist.tile([128, n_pairs, seq], DT, tag="q")
            k_sb = persist.tile([128, n_pairs, seq], DT, tag="k")
            v_sb = persist.tile([128, ntt_all, 2 * n_pairs, 65], DT, tag="v")
            af_sb = persist.tile([128, n_dch, max(seq - 512, 512)], DT, tag="af")
            af3 = []
            for p in range(n_pairs):
                t3 = persist.tile([128, 2, 512], DT, tag=f"af3_{p}")
                af3.append(t3)
            m_sb = persist.tile([128, 128], DT, tag="m")
            wo_sb = persist.tile([128, n_dch, 512], DT, tag="wo")
            wq_sb = persist.tile([128, N_MCH, hw], DT, tag="wq")
            wk_sb = persist.tile([128, N_MCH, hw], DT, tag="wk")
            wv_sb = persist.tile([128, N_MCH, hw], DT, tag="wv")
            xt = []
            for st in range(nst):
                t = persist.tile([128, N_MCH, 512], DT, tag=f"xt{st}")
                xt.append(t)

            xT_v = xT[:].rearrange("(c p) s -> p c s", p=128)
            # spread input loads across per-engine DMA queues so the
            # critical tiles (xt0, wv, wq) land concurrently
            nc.sync.dma_start(out=xt[0][:], in_=xT_v[:, :, 0:512])
            nc.scalar.dma_start(
                out=wv_sb[:], in_=wv[:].rearrange("(c p) n -> p c n", p=128))
            nc.gpsimd.dma_start(
                out=wq_sb[:], in_=wq[:].rearrange("(c p) n -> p c n", p=128))
            nc.scalar.dma_start(
                out=wk_sb[:], in_=wk[:].rearrange("(c p) n -> p c n", p=128))
            if nst > 1:
                nc.gpsimd.dma_start(
                    out=xt[1][:], in_=xT_v[:, :, 512:1024])
            for st in range(2, nst):
                nc.sync.dma_start(
                    out=xt[st][:], in_=xT_v[:, :, st * 512:(st + 1) * 512])
            nc.gpsimd.dma_start(out=m_sb[:], in_=mask128[:])
            nc.sync.dma_start(
                out=wo_sb[:], in_=wo[:].rearrange("(c p) n -> p c n", p=128))
            nc.vector.memset(v_sb[:, :, :, 64], 1.0)
            # PE warm-up during the input-load window: dummy matmuls on a
            # memset tile keep the HAM busy-window open (no DMA dependency)
            warm = persist.tile([128, 512], DT, tag="warm")
            nc.vector.memset(warm[:], 0.0)
            for wi in range(6):
                wps = ps_s.tile([128, 2, 512], F32, tag="sc",
                                name=f"warm{wi}")
                for wj in range(12):
                    nc.tensor.matmul(
                        wps[:, wj % 2, :],
                        lhsT=warm[:, 0:128], rhs=warm[:],
                        start=True, stop=True)

            yT_v = yT[:].rearrange("(t p) s -> p t s", p=128)

            # ---- emission helpers (each returns a closure doing one
            # PE-dense psum-group; used to fill PE during attention) ----
            def vproj_group(tt):
                def go():
                    st, r = tt // 4, tt % 4
                    ps = pp.tile([128, hw], F32, tag="proj", name=f"psv{tt}")
                    for c in range(N_MCH):
                        nc.tensor.matmul(
                            ps[:],
                            lhsT=xt[st][:, c, r * 128:(r + 1) * 128],
                            rhs=wv_sb[:, c, 0:hw],
                            start=(c == 0), stop=(c == N_MCH - 1))
                    nc.any.tensor_copy(
                        v_sb[:, tt, :, 0:64],
                        ps[:].rearrange("p (h k) -> p h k", k=64))
                return go

            def qkproj_group(pair, st, which):
                def go():
                    w_sb, dst = ((wq_sb, q_sb), (wk_sb, k_sb))[which]
                    ps = pp.tile([128, 512], F32, tag="proj",
                                 name=f"psqk{pair}_{st}_{which}")
                    for c in range(N_MCH):
                        nc.tensor.matmul(
                            ps[:],
                            lhsT=w_sb[:, c, pair * 128:(pair + 1) * 128],
                            rhs=xt[st][:, c, :],
                            start=(c == 0), stop=(c == N_MCH - 1))
                    nc.any.tensor_copy(
                        dst[:, pair, st * 512:(st + 1) * 512], ps[:])
                return go

            def outproj_group(mt, st, chunk_order=None):
                def go():
                    ps = pp.tile([128, 512], F32, tag="proj",
                                 name=f"pso{mt}_{st}")
                    order = chunk_order or list(range(n_dch))
                    last = st == nst - 1
                    for i, c in enumerate(order):
                        g, p = c // n_pairs, c % n_pairs
                        rhs = (af3[p][:, g, :] if last
                               else af_sb[:, c, st * 512:(st + 1) * 512])
                        nc.tensor.matmul(
                            ps[:],
                            lhsT=wo_sb[:, c, mt * 128:(mt + 1) * 128],
                            rhs=rhs,
                            start=(i == 0), stop=(i == n_dch - 1))
                    yc = ycp.tile([128, 512], F32, tag="yc", name=f"yc{mt}_{st}")
                    nc.any.tensor_copy(yc[:], ps[:])
                    nc.sync.dma_start(
                        out=yT_v[:, mt, st * 512:(st + 1) * 512], in_=yc[:])
                return go

            def proj_groups_for_st(st):
                gs = []
                for tt in range(4 * st, 4 * st + 4):
                    gs.append(vproj_group(tt))
                for pair in range(n_pairs):
                    for which in range(2):
                        gs.append(qkproj_group(pair, st, which))
                return gs

            if debug_taps:
                dpt = nc.dram_tensor("dpt", [4, 128, 2, 512], DT,
                                     kind="ExternalOutput")
                dav = nc.dram_tensor("dav", [2, 65, 512], F32,
                                     kind="ExternalOutput")
                dr = nc.dram_tensor("dr", [1, 2, 512], F32,
                                    kind="ExternalOutput")
                dbb = nc.dram_tensor("dbb", [64, 2, 512], F32,
                                     kind="ExternalOutput")

            # ---- attention for one (pair, st), software-pipelined ----
            def attention(pair, st, filler, stage, pace):
                ntt = 4 * st + 4
                av0 = ps_av.tile([65, 512], F32, tag="av",
                                 name=f"av0_{pair}_{st}")
                av1 = ps_av.tile([65, 512], F32, tag="av",
                                 name=f"av1_{pair}_{st}")
                av = [av0, av1]
                pts = {}

                def scores_and_exp(tt):
                    ps = ps_s.tile([128, 2, 512], F32, tag="sc",
                                   name=f"sc{pair}_{st}_{tt}")
                    for h in range(2):
                        lo = h * 64
                        nc.tensor.matmul(
                            ps[:, h, :],
                            lhsT=k_sb[lo:lo + 64, pair,
                                      tt * 128:(tt + 1) * 128],
                            rhs=q_sb[lo:lo + 64, pair,
                                     st * 512:(st + 1) * 512],
                            start=True, stop=True)
                    pt = p_pool.tile([128, 2, 512], DT, tag="pt",
                                     name=f"pt{pair}_{st}_{tt}")
                    kk = tt - 4 * st
                    if kk < 0:
                        nc.scalar.activation(pt[:], ps[:], EXP, scale=0.125)
                    else:
                        # diagonal: zero the fully-masked cols, exp the rest,
                        # triangular mask on the boundary 128-col block
                        nc.scalar.activation(
                            pt[:, :, kk * 128:512],
                            ps[:, :, kk * 128:512], EXP, scale=0.125)
                        for h in range(2):
                            nc.vector.tensor_mul(
                                pt[:, h, kk * 128:(kk + 1) * 128],
                                pt[:, h, kk * 128:(kk + 1) * 128],
                                m_sb[:])
                    if debug_taps and pair == 0 and st == 0:
                        nc.sync.dma_start(out=dpt[tt], in_=pt[:])
                    pts[tt] = pt

                def pv(tt):
                    pt = pts.pop(tt)
                    kk = tt - 4 * st
                    f0 = kk * 128 if kk > 0 else 0
                    for h in range(2):
                        nc.tensor.matmul(
                            av[h][:, f0:512],
                            lhsT=v_sb[:, tt, 2 * pair + h, :],
                            rhs=pt[:, h, f0:512],
                            start=(tt == 0), stop=(tt == ntt - 1))

                for tt in range(ntt + 1):
                    if tt < ntt:
                        scores_and_exp(tt)
                    if tt > 0:
                        pv(tt - 1)
                    pace["done"] += 1
                    owed = (pace["pops"] * pace["done"]) // pace["total"] \
                        - pace["popped"]
                    while filler and owed > 0:
                        filler.pop(0)()
                        pace["popped"] += 1
                        owed -= 1

                if debug_taps and pair == 0 and st == 0:
                    for h in range(2):
                        avc = nrm.tile([65, 512], F32, tag="avc",
                                       name=f"avc{h}")
                        nc.vector.tensor_copy(avc[:], av[h][:])
                        nc.sync.dma_start(out=dav[h], in_=avc[:])
                # normalize: a = av[0:64] * (1/denom); denom row (psum
                # partition 64) -> sbuf -> DMA to partition 0 (the custom-DVE
                # recip and gpsimd broadcast only read partition 0 correctly)
                den = nrm.tile([65, 2, 512], DT, tag="den",
                               name=f"den{pair}_{st}")
                for h in range(2):
                    nc.vector.tensor_copy(den[:, h, :], av[h][:])
                den0 = nrm.tile([1, 2, 512], F32, tag="den0",
                                name=f"den0_{pair}_{st}")
                nc.gpsimd.dma_start(out=den0[:], in_=den[64:65, :, :])
                r = nrm.tile([1, 2, 512], F32, tag="r", name=f"r{pair}_{st}")
                nc.vector.reciprocal_approx_fast(r[:], den0[:])
                bb = nrm.tile([64, 2, 512], F32, tag="b", name=f"bb{pair}_{st}")
                nc.gpsimd.partition_broadcast(bb[:], r[:])
                if debug_taps and pair == 0 and st == 0:
                    nc.sync.dma_start(out=dr[:], in_=r[:])
                    nc.sync.dma_start(out=dbb[:], in_=bb[:])
                for h in range(2):
                    nc.vector.tensor_mul(
                        stage[:, h, pair, :],
                        den[0:64, h, :], bb[:, h, :])

            # ---------------- main s-tile-outer schedule ----------------
            pending = proj_groups_for_st(0)
            while pending:
                pending.pop(0)()
            for st in range(nst):
                filler = []
                if st + 1 < nst:
                    filler += proj_groups_for_st(st + 1)
                if st >= 1:
                    for mt in range(4):
                        filler.append(outproj_group(mt, st - 1))
                stage = stg.tile([64, 2, n_pairs, 512], DT, tag="stage",
                                 name=f"stage{st}")
                total_iters = n_pairs * (4 * st + 5)
                pace = {"total": total_iters, "done": 0,
                        "pops": len(filler), "popped": 0}
                for pair in range(n_pairs):
                    attention(pair, st, filler, stage, pace)
                    if st == nst - 1:
                        for h in range(2):
                            nc.sync.dma_start(
                                out=ag_in_l[pair, h],
                                in_=stage[:, h, pair, :])
                        nc.gpsimd.collective_compute(
                            "AllGather",
                            mybir.AluOpType.bypass,
                            replica_groups=REPLICA_GROUPS,
                            ins=[ag_in_l[pair].opt()],
                            outs=[ag_out_l[pair].opt()],
                        )
                        for g in range(2):
                            for h in range(2):
                                nc.sync.dma_start(
                                    out=af3[pair][h * 64:(h + 1) * 64, g, :],
                                    in_=ag_out_l[pair, g, h])
                while filler:
                    filler.pop(0)()
                # exchange this s-tile's attention columns
                if st < nst - 1:
                    for h in range(2):
                        nc.sync.dma_start(
                            out=ag_in[st, h], in_=stage[:, h, :, :])
                    nc.gpsimd.collective_compute(
                        "AllGather",
                        mybir.AluOpType.bypass,
                        replica_groups=REPLICA_GROUPS,
                        ins=[ag_in[st].opt()],
                        outs=[ag_out[st].opt()],
                    )
                    for g in range(2):
                        for h in range(2):
                            nc.sync.dma_start(
                                out=af_sb[h * 64:(h + 1) * 64,
                                          g * n_pairs:(g + 1) * n_pairs,
                                          st * 512:(st + 1) * 512],
                                in_=ag_out[st, g, h])
            pair_major = [g * n_pairs + p for p in range(n_pairs)
                          for g in range(2)]
            for mt in range(4):
                outproj_group(mt, nst - 1, chunk_order=pair_major)()

            if debug_taps:
                dq = nc.dram_tensor("dq", [128, n_pairs, seq], DT,
                                    kind="ExternalOutput")
                dk = nc.dram_tensor("dk", [128, n_pairs, seq], DT,
                                    kind="ExternalOutput")
                dv = nc.dram_tensor("dv", [128, ntt_all, 2 * n_pairs, 65], DT,
                                    kind="ExternalOutput")
                daf = nc.dram_tensor("daf", [128, n_dch, seq], DT,
                                     kind="ExternalOutput")
                for dst, src in ((dq, q_sb), (dk, k_sb), (dv, v_sb),
                                 (daf, af_sb)):
                    nc.sync.dma_start(out=dst[:], in_=src[:])
    nc.compile()
    return nc


def _make_mask128():
    p = np.arange(128)[:, None]
    f = np.arange(128)[None, :]
    return (p <= f).astype(BF16)


_NC_CACHE = {}


def _get_nc(seq=S, n_pairs=N_PAIRS):
    key = (seq, n_pairs)
    if key not in _NC_CACHE:
        _NC_CACHE[key] = build_nc(seq, n_pairs)
    return _NC_CACHE[key]


def make_in_maps(x, w_qkv, w_o):
    masks = _make_mask128()
    in_maps = []
    for c in range(N_CORES):
        b, hg = c // 2, c % 2
        heads = slice(hg * 8, hg * 8 + 8)
        in_maps.append({
            "xT": np.ascontiguousarray(x[b].T).astype(BF16),
            "wq": np.ascontiguousarray(
                w_qkv[0, heads].reshape(512, DM).T).astype(BF16),
            "wk": np.ascontiguousarray(
                w_qkv[1, heads].reshape(512, DM).T).astype(BF16),
            "wv": np.ascontiguousarray(
                w_qkv[2, heads].reshape(512, DM).T).astype(BF16),
            "wo": np.ascontiguousarray(
                w_o[hg * 512:(hg + 1) * 512, :].T).astype(BF16),
            "mask128": masks,
        })
    return in_maps


def kernel(x, w_qkv, w_o):
    x = np.asarray(x, dtype=np.float32)
    w_qkv = np.asarray(w_qkv, dtype=np.float32)
    w_o = np.asarray(w_o, dtype=np.float32)

    nc = _get_nc()
    in_maps = make_in_maps(x, w_qkv, w_o)
    res = run_bass_kernel_spmd(nc, in_maps, list(range(N_CORES)), trace=False)

    y = np.empty((B, S, DM), dtype=np.float32)
    for c in range(N_CORES):
        b, hg = c // 2, c % 2
        y[b, :, hg * 512:(hg + 1) * 512] = res.results[c]["yT"].T
    return y
